# revision 4
# baseline (speedup 1.0000x reference)
"""Trainium2 Bass kernel for nn_MixquantLinear: O = ((dequant4(V) * S) @ dequant4(U)).T.

Output O is [4096, 4096] fp32 built from the GPTQ weights (activation x is dead
code). Sharding: 4 (out rows) x 2 (out cols) -> 8 cores, no collectives.

fp8 (e4m3, DoubleRow perf mode, 2x PE rate) matmul pipeline per core:
  - host XORs packed nibbles with 0x8 so a (shl, asr) unpack yields s = q-8
    (centered int4, exact in fp8; halves V-side rounding variance)
  - V rhs = fp8(av * s), av = sv*S*1024; the zero-point part (exact, fp32) is
    folded into a host-computed rank-16 correction C[o, gi] added at flush
  - U lhsT = fp8(fp16(fp16(au*s) + du)) built with broadcast (stride-0)
    tensor_tensor ops, PE-transposed in fp16, fp8-converted in the PSUM copy
  - DoubleRow matmuls: k = ksub*128 + p, two k-subtiles per instruction
  - flush: out = psum * 2^-20 + C (scalar_tensor_tensor from PSUM)
N8 = number of fp8 k-tiles (rest fp16) trades accuracy vs PE time.
"""

import numpy as np

try:
    import ml_dtypes
    _E4M3 = ml_dtypes.float8_e4m3
except Exception:  # pragma: no cover
    _E4M3 = None

import concourse.bass as bass  # noqa: F401
import concourse.mybir as mybir
import concourse.tile as tile
from concourse import bacc
from concourse.bass_utils import run_bass_kernel_spmd
from concourse.masks import make_identity

IN_SIZE = 4096
OUT_SIZE = 4096
RANK = 1024
PACK = 8
P_O = 4
P_I = 2
O_SL = OUT_SIZE // P_O    # 1024
I_SL = IN_SIZE // P_I     # 2048
N_CORES = P_O * P_I
KT = 8                    # k tiles of 128
OT = 8                    # o tiles of 128
IC = 4                    # i chunks of 512
N_STRIPS = 2
STRIP = I_SL // N_STRIPS  # 1024

N8 = 8                    # fp8 k-tiles (even); rest fp16
SCALE = 1024.0
ISCALE2 = float(2.0 ** -20)
XOR_WORD = np.int32(-2004318072)  # 0x88888888

F8 = mybir.dt.float8e4
F16 = mybir.dt.float16
F32 = mybir.dt.float32
I32 = mybir.dt.int32
Alu = mybir.AluOpType
Act = mybir.ActivationFunctionType
DRMODE = mybir.MatmulPerfMode.DoubleRow

_NC_CACHE = {}
TRACE = False
LAST_RESULTS = None


def _build_nc(n8):
    kt16 = KT - n8
    np2 = n8 // 2
    nc = bacc.Bacc("TRN2", target_bir_lowering=False)

    qvt = nc.dram_tensor("qvt", [128, N_STRIPS * KT * 128], I32, kind="ExternalInput")
    av_d = nc.dram_tensor("av", [128, N_STRIPS * KT * 8], F32, kind="ExternalInput")
    qut = nc.dram_tensor("qut", [128, OT * 128], I32, kind="ExternalInput")
    au_d = nc.dram_tensor("au", [128, OT * KT], F32, kind="ExternalInput")
    du_d = nc.dram_tensor("du", [128, OT * KT], F32, kind="ExternalInput")
    cc_d = nc.dram_tensor("cc", [128, OT * 16], F32, kind="ExternalInput")
    out = nc.dram_tensor("out", [O_SL, I_SL], F32, kind="ExternalOutput")

    with tile.TileContext(nc) as tc:
        with (
            tc.tile_pool(name="const", bufs=1) as cp,
            tc.tile_pool(name="outsb", bufs=8) as outp,
        ):
            qvt_sb = cp.tile([128, N_STRIPS * KT * 128], I32, tag="qvt")
            av_sb = cp.tile([128, N_STRIPS * KT * 8], F32, tag="av")
            qut_sb = cp.tile([128, OT * 128], I32, tag="qut")
            au_sb = cp.tile([128, OT * KT], F32, tag="au")
            du_sb = cp.tile([128, OT * KT], F32, tag="du")
            cc_sb = cp.tile([128, OT * 16], F32, tag="cc")
            nibu = cp.tile([128, OT * RANK], I32, tag="nibu")
            uw16 = cp.tile([128, OT * RANK], F16, tag="uw16")
            u16 = cp.tile([128, OT * RANK], F16, tag="u16")
            nibv = [cp.tile([128, KT * STRIP], I32, tag=f"nibv{s}", name=f"nibv{s}")
                    for s in range(N_STRIPS)]
            id16 = cp.tile([128, 128], F16, tag="id16")
            if n8:
                rhs8 = cp.tile([128, n8, I_SL], F8, tag="rhs8")
                lhsT8 = cp.tile([128, n8, O_SL], F8, tag="lhsT8")
            if kt16:
                rhs16 = cp.tile([128, kt16, I_SL], F16, tag="rhs16")
                lhsT16 = cp.tile([128, kt16, O_SL], F16, tag="lhsT16")

            make_identity(nc, id16[:])
            nc.sync.dma_start(out=qut_sb[:], in_=qut[:])
            nc.sync.dma_start(out=au_sb[:], in_=au_d[:])
            nc.sync.dma_start(out=du_sb[:], in_=du_d[:])
            nc.sync.dma_start(out=av_sb[:], in_=av_d[:])
            nc.sync.dma_start(out=cc_sb[:], in_=cc_d[:])
            half = KT * 128
            nc.sync.dma_start(out=qvt_sb[:, 0:half], in_=qvt[:, 0:half])
            nc.sync.dma_start(out=qvt_sb[:, half:2 * half], in_=qvt[:, half:2 * half])

            # ---- U: signed unpack (DVE), 2-pass broadcast affine (DVE) ----
            nu_r = nibu[:].rearrange("p (w j) -> p w j", j=PACK)
            for j in range(PACK):
                nc.vector.tensor_scalar(
                    out=nu_r[:, :, j], in0=qut_sb[:],
                    scalar1=28 - 4 * j, scalar2=28,
                    op0=Alu.logical_shift_left, op1=Alu.arith_shift_right)
            nu_g = nibu[:].rearrange("p (g c) -> p g c", c=128)
            uw_g = uw16[:].rearrange("p (g c) -> p g c", c=128)
            u16_g = u16[:].rearrange("p (g c) -> p g c", c=128)
            au_b = au_sb[:].unsqueeze(2).broadcast_to([128, OT * KT, 128])
            du_b = du_sb[:].unsqueeze(2).broadcast_to([128, OT * KT, 128])
            nc.vector.tensor_tensor(uw_g, nu_g, au_b, Alu.mult)
            nc.vector.tensor_tensor(u16_g, uw_g, du_b, Alu.add)

            # ---- U transposes (fp16 via PSUM) + convert-copies ----
            with tc.tile_pool(name="tps", bufs=2, space="PSUM") as tps:
                for t in range(OT):
                    for kq in range(2):
                        pt = tps.tile([128, 4096], F16, tag="tp", name="tp")
                        for kk in range(4):
                            rt = kq * 4 + kk
                            nc.tensor.transpose(
                                pt[:, kk * 1024:kk * 1024 + 128],
                                u16[:, t * RANK + rt * 128:t * RANK + (rt + 1) * 128],
                                id16[:])
                        src = pt.rearrange("p (x c) -> p x c", x=4)[:, :, :128]
                        # copy pairs so n8 in {0,2,4,6,8} works
                        for hp in range(2):
                            rt0 = kq * 4 + hp * 2
                            s2 = src[:, hp * 2:hp * 2 + 2, :]
                            if rt0 < n8:
                                dst = lhsT8[:, rt0:rt0 + 2, t * 128:(t + 1) * 128]
                            else:
                                dst = lhsT16[:, rt0 - n8:rt0 - n8 + 2,
                                             t * 128:(t + 1) * 128]
                            nc.scalar.copy(dst, s2)

            # ---- V per strip: signed unpack + broadcast affine ----
            def v_strip(st):
                words = qvt_sb[:, st * half:(st + 1) * half]
                nv_r = nibv[st][:].rearrange("p (w j) -> p w j", j=PACK)
                for j in range(PACK):
                    nc.vector.tensor_scalar(
                        out=nv_r[:, :, j], in0=words,
                        scalar1=28 - 4 * j, scalar2=28,
                        op0=Alu.logical_shift_left, op1=Alu.arith_shift_right)
                for rt in range(KT):
                    src = nibv[st][:, rt * STRIP:(rt + 1) * STRIP] \
                        .rearrange("p (g c) -> p g c", c=128)
                    a_sl = av_sb[:, (st * KT + rt) * 8:(st * KT + rt + 1) * 8]
                    a_b = a_sl.unsqueeze(2).broadcast_to([128, 8, 128])
                    if rt < n8:
                        dst = rhs8[:, rt, st * STRIP:(st + 1) * STRIP]
                    else:
                        dst = rhs16[:, rt - n8, st * STRIP:(st + 1) * STRIP]
                    dst = dst.rearrange("p (g c) -> p g c", c=128)
                    nc.vector.tensor_tensor(dst, src, a_b, Alu.mult)

            def mm_wave(ic):
                base = ic * 512
                for ot in range(OT):
                    pt = mps.tile([128, 512], F32, tag="mm", name="mm")
                    for kp in range(np2):
                        nc.tensor.matmul(
                            pt[:],
                            lhsT8[:, 2 * kp:2 * kp + 2, ot * 128:(ot + 1) * 128],
                            rhs8[:, 2 * kp:2 * kp + 2, base:base + 512],
                            start=(kp == 0), stop=(kp == np2 - 1 and kt16 == 0),
                            perf_mode=DRMODE, skip_group_check=True)
                    for k6 in range(kt16):
                        nc.tensor.matmul(
                            pt[:],
                            lhsT16[:, k6, ot * 128:(ot + 1) * 128],
                            rhs16[:, k6, base:base + 512],
                            start=(n8 == 0 and k6 == 0), stop=(k6 == kt16 - 1),
                            skip_group_check=True)
                    ot_t = outp.tile([128, 512], F32, tag="ot", name="ot")
                    cc_b = cc_sb[:, ot * 16 + ic * 4:ot * 16 + (ic + 1) * 4] \
                        .unsqueeze(2).broadcast_to([128, 4, 128])
                    nc.vector.scalar_tensor_tensor(
                        out=ot_t[:].rearrange("p (g c) -> p g c", c=128),
                        in0=pt[:].rearrange("p (g c) -> p g c", c=128),
                        scalar=ISCALE2, in1=cc_b, op0=Alu.mult, op1=Alu.add)
                    nc.sync.dma_start(
                        out=out[ot * 128:(ot + 1) * 128, base:base + 512],
                        in_=ot_t[:])

            with tc.tile_pool(name="mps", bufs=8, space="PSUM") as mps:
                v_strip(0)
                mm_wave(0)
                v_strip(1)
                mm_wave(1)
                mm_wave(2)
                mm_wave(3)

    nc.compile()
    return nc


def _unpack_cols(qz):
    shifts = np.arange(PACK, dtype=np.int32) * 4
    G, W = qz.shape
    return ((qz[:, :, None] >> shifts[None, None, :]) & 15).reshape(G, W * PACK)


def _cast8(x):
    return x.astype(_E4M3).astype(np.float32)


def _cast16(x):
    return x.astype(np.float16).astype(np.float32)


def _host_prep(qweight_V, qzeros_V, scales_V, qweight_U, qzeros_U, scales_U, S,
               n8):
    zv_full = _unpack_cols(qzeros_V).astype(np.float32) + 1.0   # [32, 1024]
    zu_full = _unpack_cols(qzeros_U).astype(np.float32) + 1.0   # [8, 4096]
    qv_x = qweight_V ^ XOR_WORD
    qu_x = qweight_U ^ XOR_WORD

    # host model of U lhsT values for the C table (per k-tile, fp16 route)
    shifts = np.arange(PACK, dtype=np.int32) * 4
    qu_full = (((qweight_U[:, None, :] >> shifts[None, :, None]) & 15)
               .reshape(RANK, OUT_SIZE).astype(np.float32))     # [r, out]
    au_full = (scales_U * SCALE).astype(np.float32)             # [8, out]
    du_full = (au_full * (8.0 - zu_full)).astype(np.float32)    # [8, out]
    lhs_val = np.empty((RANK, OUT_SIZE), np.float32)
    for t in range(KT):
        sl = slice(t * 128, (t + 1) * 128)
        p1 = _cast16((qu_full[sl] - 8.0) * au_full[t][None, :])
        p2 = _cast16(p1 + du_full[t][None, :])
        lhs_val[sl] = _cast8(p2) if t < n8 else p2

    av_full = (scales_V * S[None, :] * SCALE).astype(np.float32)   # [32, r]
    dv_full = (av_full * (8.0 - zv_full)).astype(np.float32)       # [32, r]

    in_maps = []
    for c in range(N_CORES):
        a, b = divmod(c, P_I)
        # V packed words, layout [p, (st, rt, w)]
        qv = qv_x[b * (I_SL // PACK):(b + 1) * (I_SL // PACK), :]  # [256 w, 1024 r]
        qvt_h = np.ascontiguousarray(
            qv.T.reshape(KT, 128, N_STRIPS, 128).transpose(1, 2, 0, 3)
            .reshape(128, -1))
        # av layout [p, (st, rt, g)]
        avc = av_full[b * 16:(b + 1) * 16, :]                      # [16 gi, 1024 r]
        av_h = np.ascontiguousarray(
            avc.T.reshape(KT, 128, N_STRIPS, 8).transpose(1, 2, 0, 3)
            .reshape(128, -1))
        # U packed words, layout [p(o), (t, w)]
        qu = qu_x[:, a * O_SL:(a + 1) * O_SL]                      # [128 w, 1024 o]
        qut_h = np.ascontiguousarray(
            qu.T.reshape(OT, 128, 128).transpose(1, 0, 2).reshape(128, -1))
        # au/du layout [p(o), (t, g)]
        auc = au_full[:, a * O_SL:(a + 1) * O_SL]                  # [8 g, 1024 o]
        duc = du_full[:, a * O_SL:(a + 1) * O_SL]
        au_h = np.ascontiguousarray(
            auc.T.reshape(OT, 128, KT).transpose(1, 0, 2).reshape(128, -1))
        du_h = np.ascontiguousarray(
            duc.T.reshape(OT, 128, KT).transpose(1, 0, 2).reshape(128, -1))
        # C[o, gi] = sum_r lhs_val[r, o] * dv[gi, r], scaled by 2^-20
        lv = lhs_val[:, a * O_SL:(a + 1) * O_SL]                   # [r, 1024 o]
        dvc = dv_full[b * 16:(b + 1) * 16, :]                      # [16 gi, r]
        ccc = (lv.T @ dvc.T) * ISCALE2                             # [1024 o, 16]
        cc_h = np.ascontiguousarray(
            ccc.reshape(OT, 128, 16).transpose(1, 0, 2).reshape(128, -1)
            .astype(np.float32))
        in_maps.append({"qvt": qvt_h, "av": av_h, "qut": qut_h,
                        "au": au_h, "du": du_h, "cc": cc_h})
    return in_maps


def kernel(x, qweight_V, qzeros_V, scales_V, g_idx_V,
           qweight_U, qzeros_U, scales_U, g_idx_U, S, **_unused):
    global LAST_RESULTS
    qweight_V = np.asarray(qweight_V, dtype=np.int32)
    qzeros_V = np.asarray(qzeros_V, dtype=np.int32)
    scales_V = np.asarray(scales_V, dtype=np.float32)
    qweight_U = np.asarray(qweight_U, dtype=np.int32)
    qzeros_U = np.asarray(qzeros_U, dtype=np.int32)
    scales_U = np.asarray(scales_U, dtype=np.float32)
    S = np.asarray(S, dtype=np.float32)

    if N8 not in _NC_CACHE:
        _NC_CACHE[N8] = _build_nc(N8)
    nc = _NC_CACHE[N8]

    in_maps = _host_prep(qweight_V, qzeros_V, scales_V,
                         qweight_U, qzeros_U, scales_U, S, N8)
    res = run_bass_kernel_spmd(nc, in_maps, core_ids=list(range(N_CORES)),
                               trace=TRACE)
    LAST_RESULTS = res

    O = np.empty((OUT_SIZE, IN_SIZE), dtype=np.float32)
    for c in range(N_CORES):
        a, b = divmod(c, P_I)
        O[a * O_SL:(a + 1) * O_SL, b * I_SL:(b + 1) * I_SL] = res.results[c]["out"]
    return O
